# revision 2
# baseline (speedup 1.0000x reference)
"""Trainium2 Bass kernel for nn_Capsule, v3: fine-grained stream + short routing.

Math: einsum 'nco,pbo->bno' factorizes as xp[b,n,o] = W[n,o]*X[b,o] with
W = caps_weights.sum(c), X = x.sum(p); then a 3-iteration routing loop on
the tiny per-batch state.

v3 changes vs v2 (95us baseline):
  - x streams as 20 chunk DMAs (16 x 1MB for slabs 0-7 split in half-batches,
    4 x 0.5MB for slab 9) alternating the sync/scalar HWDGE rings, so the
    one-hot reduction matmuls consume each chunk as it lands instead of in
    multi-MB bursts; the last dependency unit is 8 batch rows.
  - reduction is all 128-wide single matmuls (observed 60ns cadence) into
    one psum tile [32,128]; no strided pair reduce.
  - routing keeps b-on-partitions but: transposes via DVE 32x32 stream
    transposes (no PE/psum round trip), logits accumulate IN PSUM across
    iterations (start/stop split), softmax sum comes free from a ones
    column appended to W (matmul [64]x[129]), and the squash scale is
    sqrt(nsq)*rden*rsum^2 with only two ACT ops (Ln, Exp) on the chain.
"""

import numpy as np

# ---- problem constants (hardcoded per contract) ----
P_TOT = 1152
BATCH = 256
O = 128
N_CAPS = 64
CAPS_DIM = 16
ITERATIONS = 3
N_CORES = 8
B_LOC = BATCH // N_CORES          # 32 batch elements per core
PT = P_TOT // 128                 # 9 p-slabs

_cache = {}


def _pin_act_table():
    """Force every ACT function onto the one table containing Exp+Ln, so a
    single ACT_TABLE_LOAD suffices."""
    import functools
    import concourse.hw_specs as hw_specs
    import concourse.bacc as bacc_mod

    if getattr(hw_specs.get_activation_tables, "_capsule_pinned", False):
        return
    orig = hw_specs.get_activation_tables

    @functools.cache
    def pinned(module_arch):
        tabs = orig(module_arch)
        keep = None
        for name, fns in tabs.items():
            names = {f.name for f in fns}
            if {"Exp", "Ln", "Square", "Copy", "Identity"} <= names:
                keep = name
                break
        if keep is None:
            return tabs
        return {n: (fns if n == keep else type(fns)()) for n, fns in tabs.items()}

    pinned._capsule_pinned = True
    hw_specs.get_activation_tables = pinned
    bacc_mod.get_activation_tables = pinned


def _build():
    _pin_act_table()
    import concourse.bacc as bacc
    import concourse.tile as tile
    import concourse.mybir as mybir
    from concourse.masks import make_identity

    f32 = mybir.dt.float32
    f32r = mybir.dt.float32r
    AX = mybir.AxisListType
    AF = mybir.ActivationFunctionType
    OP = mybir.AluOpType

    nc = bacc.Bacc(None, target_bir_lowering=False)

    # x declared f32r: same bytes as fp32; feeds the fast fp32r matmul path.
    x_in = nc.dram_tensor("x", [P_TOT, B_LOC, O], f32r, kind="ExternalInput")
    w_in = nc.dram_tensor("caps_weights", [N_CAPS, CAPS_DIM, O], f32,
                          kind="ExternalInput")
    out_d = nc.dram_tensor("out", [B_LOC, O], f32, kind="ExternalOutput")

    xv = x_in.rearrange("(t p) b o -> t p b o", p=128)   # (9, 128, 32, 128)

    # chunk list: (slab t, batch offset b0, n batches nb)
    chunks = [(t, h * 16, 16) for t in range(8) for h in (0, 1)]
    chunks += [(8, qq * 8, 8) for qq in range(4)]

    with tile.TileContext(nc) as tc:
        with (
            tc.tile_pool(name="xin", bufs=1) as xpool,
            tc.tile_pool(name="wrk", bufs=1) as wrk,
            tc.tile_pool(name="small", bufs=1) as small,
            tc.tile_pool(name="ps", bufs=1, space="PSUM") as ps,
        ):
            xt = [xpool.tile([128, nb * O], f32r, tag=f"xc{i}", name=f"xc{i}")
                  for i, (t, b0, nb) in enumerate(chunks)]

            # ---- DMA issue: w halves first (one per ring, byte balance),
            # chunks alternate sync/scalar
            w_sb = wrk.tile([N_CAPS, CAPS_DIM * O], f32)
            wv = w_in.rearrange("n c o -> n (c o)")
            nc.scalar.dma_start(w_sb[:32, :], wv[:32, :])
            nc.sync.dma_start(w_sb[32:, :], wv[32:, :])
            for i, (t, b0, nb) in enumerate(chunks):
                eng = nc.sync if i % 2 == 0 else nc.scalar
                vvv = xt[i][:].rearrange("p (b o) -> p b o", b=nb)
                eng.dma_start(vvv, xv[t][:, b0:b0 + nb, :])

            # one-hot stationary source: (128, 63) with ones in column 31;
            # zpat(b) = cols [31-b, 63-b) -> one-hot column b.
            zpat_f = small.tile([128, 2 * B_LOC - 1], f32)
            nc.gpsimd.memset(zpat_f[:], 0.0)
            nc.gpsimd.memset(zpat_f[:, B_LOC - 1:B_LOC], 1.0)

            def zpat(b):
                return zpat_f[:, B_LOC - 1 - b: 2 * B_LOC - 1 - b].bitcast(f32r)

            ident = small.tile([128, 128], f32)
            make_identity(nc, ident[:])
            unif = small.tile([N_CAPS, B_LOC], f32)
            nc.gpsimd.memset(unif[:], 1.0 / N_CAPS)

            # ---- capsule weight prep (overlaps the x stream) ----
            # fold 16 caps_dim rows in place; result W[n,o] -> w_no1[:, :128],
            # with a ones column at [:, 128] for the free softmax sum.
            nc.vector.tensor_tensor(w_sb[:, :8 * O], w_sb[:, :8 * O],
                                    w_sb[:, 8 * O:], OP.add)
            nc.vector.tensor_tensor(w_sb[:, :4 * O], w_sb[:, :4 * O],
                                    w_sb[:, 4 * O:8 * O], OP.add)
            nc.vector.tensor_tensor(w_sb[:, :2 * O], w_sb[:, :2 * O],
                                    w_sb[:, 2 * O:4 * O], OP.add)
            w_no1 = wrk.tile([N_CAPS, O + 1], f32)
            nc.gpsimd.memset(w_no1[:, O:O + 1], 1.0)
            nc.vector.tensor_tensor(w_no1[:, :O], w_sb[:, :O],
                                    w_sb[:, O:2 * O], OP.add)

            # W^T[o,n] via PE transpose; S0 = (1/64) sum_n W[n,:] per row.
            ps_wt = ps.tile([O, N_CAPS], f32, tag="ps_wt")
            nc.tensor.transpose(ps_wt[:], w_no1[:, :O],
                                ident[:N_CAPS, :N_CAPS])
            wt_on = wrk.tile([O, N_CAPS], f32)
            nc.vector.tensor_copy(wt_on[:], ps_wt[:])
            ps_s0 = ps.tile([B_LOC, O], f32, tag="ps_s0")
            nc.tensor.matmul(ps_s0[:], unif[:], w_no1[:, :O],
                             start=True, stop=True)
            s0_sb = wrk.tile([B_LOC, O], f32)
            nc.vector.tensor_copy(s0_sb[:], ps_s0[:])

            # ---- reduction: ps_x[b,o] = sum_p x[p,b,o], 288 matmuls ----
            ps_x = ps.tile([B_LOC, O], f32, tag="ps_x")
            last = len(chunks) - 1
            for i, (t, b0, nb) in enumerate(chunks):
                for j in range(nb):
                    nc.tensor.matmul(
                        ps_x[:], zpat(b0 + j),
                        xt[i][:, j * O:(j + 1) * O],
                        start=(i == 0 and j == 0),
                        stop=(i == last and j == nb - 1),
                        skip_group_check=True)

            # ---- routing (b on partitions; logits accumulate in psum) ----
            x32 = wrk.tile([B_LOC, O], f32)
            ue = [wrk.tile([B_LOC, O], f32, name=f"ue{i}") for i in range(3)]
            uu = [wrk.tile([B_LOC, O], f32, name=f"uu{i}") for i in range(3)]
            uxr = [wrk.tile([B_LOC, O], f32, name=f"uxr{i}") for i in range(2)]
            sqs = wrk.tile([B_LOC, O], f32)
            tb = [wrk.tile([B_LOC, O], f32, name=f"tb{i}") for i in range(2)]
            tT = [wrk.tile([O, B_LOC], f32, name=f"tT{i}") for i in range(2)]
            exT = [wrk.tile([N_CAPS, B_LOC], f32, name=f"exT{i}")
                   for i in range(2)]
            out_sb = wrk.tile([B_LOC, O], f32)

            def col(name):
                return wrk.tile([B_LOC, 1], f32, name=name)

            ps_dT = ps.tile([N_CAPS, B_LOC], f32, tag="ps_dT")
            ps_s2 = [ps.tile([B_LOC, O + 1], f32, tag=f"ps_s2_{i}",
                             name=f"ps_s2_{i}") for i in range(2)]

            for it in range(ITERATIONS):
                lastit = it == ITERATIONS - 1
                # u = coeffs@xp (normalized): keep values O(1) so nsq stays
                # inside the ACT Ln table input range (unnormalized blows up
                # to ~1e28 and the table returns garbage).
                if it == 0:
                    # S0 exact (uniform coeffs); rsum = 1 -> u = ue
                    nc.vector.tensor_tensor(uu[0][:], s0_sb[:], ps_x[:],
                                            OP.mult)
                else:
                    sp = ps_s2[it - 1]
                    rsum = col(f"rsum{it}")
                    nc.vector.reciprocal(rsum[:], sp[:, O:O + 1])
                    nc.vector.tensor_tensor(ue[it][:], sp[:, :O], x32[:],
                                            OP.mult)
                    nc.vector.tensor_scalar_mul(uu[it][:], ue[it][:],
                                                rsum[:])
                u = uu[it]
                nsq = col(f"nsq{it}")
                nc.vector.tensor_tensor(sqs[:], u[:], u[:], OP.mult)
                nc.vector.tensor_reduce(nsq[:], sqs[:], AX.X, OP.add)
                den = col(f"den{it}")
                rden = col(f"rden{it}")
                nc.vector.tensor_scalar_add(den[:], nsq[:], 1.0)
                nc.vector.reciprocal(rden[:], den[:])
                if it == 0:
                    nc.vector.tensor_tensor(uxr[0][:], u[:], ps_x[:],
                                            OP.mult)
                    # park X in SBUF for later iterations (off-chain)
                    nc.vector.tensor_copy(x32[:], ps_x[:])
                elif not lastit:
                    nc.vector.tensor_tensor(uxr[it][:], u[:], x32[:],
                                            OP.mult)

                # squash scale: m = sqrt(nsq)/(1+nsq) via Exp(0.5*Ln(nsq))
                lnq = col(f"lnq{it}")
                sqq = col(f"sqq{it}")
                nc.scalar.activation(lnq[:], nsq[:], AF.Ln)
                nc.scalar.activation(sqq[:], lnq[:], AF.Exp, scale=0.5)
                m = col(f"m{it}")
                nc.vector.tensor_tensor(m[:], sqq[:], rden[:], OP.mult)
                if lastit:
                    nc.vector.tensor_scalar_mul(out_sb[:], u[:], m[:])
                    nc.scalar.dma_start(out_d[:], out_sb[:])
                else:
                    nc.vector.tensor_scalar_mul(tb[it][:], uxr[it][:], m[:])
                    for jj in range(4):
                        nc.vector.transpose(
                            tT[it][32 * jj:32 * (jj + 1), :],
                            tb[it][:, 32 * jj:32 * (jj + 1)])
                    # logits (transposed) accumulate in psum across iters
                    nc.tensor.matmul(ps_dT[:], wt_on[:], tT[it][:],
                                     start=(it == 0),
                                     stop=(it == ITERATIONS - 2),
                                     skip_group_check=True)
                    nc.scalar.activation(exT[it][:], ps_dT[:], AF.Exp)
                    nc.tensor.matmul(ps_s2[it][:], exT[it][:], w_no1[:],
                                     start=True, stop=True)

    nc.compile()
    return nc


def run_with_results(x: np.ndarray, caps_weights: np.ndarray, **run_kwargs):
    """Run the SPMD kernel; returns (output (256,1,128), BassKernelResults)."""
    from concourse.bass_utils import run_bass_kernel_spmd

    if "nc" not in _cache:
        _cache["nc"] = _build()
    nc = _cache["nc"]

    x = np.ascontiguousarray(x, dtype=np.float32)
    caps_weights = np.ascontiguousarray(caps_weights, dtype=np.float32)

    in_maps = []
    for c in range(N_CORES):
        in_maps.append({
            "x": np.ascontiguousarray(x[:, c * B_LOC:(c + 1) * B_LOC, :]),
            "caps_weights": caps_weights,
        })
    res = run_bass_kernel_spmd(nc, in_maps, core_ids=list(range(N_CORES)),
                               **run_kwargs)
    out = np.concatenate([res.results[c]["out"] for c in range(N_CORES)],
                         axis=0)
    return out.reshape(BATCH, 1, O), res


def kernel(x: np.ndarray, caps_weights: np.ndarray) -> np.ndarray:
    out, _ = run_with_results(x, caps_weights)
    return out
